# revision 26
# baseline (speedup 1.0000x reference)
"""Trainium2 Bass kernel for nn_BitwiseTasNetBlock.

Model: 4 layers of [1x1 conv C->D, PReLU, BN, dilated depthwise conv K=3,
PReLU, BN, 1x1 conv D->C] with a residual around the whole stack.
B=8, C=128, D=512, T=8000. Training-mode BatchNorm -> stats over (batch, time).

Sharding: data-parallel over batch, one batch element per NeuronCore (8 cores).

v2 design (bf16 compute):
  - All activations and conv weights are bf16; matmuls run at 1 cycle/row on
    the PE (4x the fp32 rate). PSUM accumulation stays f32.
  - BN stats via bn_stats/bn_aggr on DVE (bf16 input), exchanged as
    (mean, mean^2+var) through a small AllGather, as in the fp32 baseline.
  - A dummy AllGather at kernel start absorbs the collectives bootstrap
    barrier (~49us) and the cold-start cost of the first real exchange.
  - BN1 affine folds into the PReLU2 activation (scale/bias APs); depthwise
    edge columns use per-edge bias variants. BN2 folds into conv2 weights
    (scaled to bf16 on device) + bias (W2 @ t2 matvec).
  - conv2 PSUM -> SBUF (+bias) runs on DVE tensor_scalar, keeping the scalar
    engine free for the PReLUs. The residual is an identity matmul into the
    last conv2 accumulation.
"""

import numpy as np
from contextlib import ExitStack

import ml_dtypes

import concourse.bass as bass
import concourse.bacc as bacc
import concourse.mybir as mybir
import concourse.tile as tile
from concourse.bass_utils import run_bass_kernel_spmd

F32 = mybir.dt.float32
BF16 = mybir.dt.bfloat16
AF = mybir.ActivationFunctionType
ALU = mybir.AluOpType

NCORES = 8
B, C, D, T, L, K = 8, 128, 512, 8000, 4, 3
G = D // 128          # 4 channel groups of 128 partitions
PAD = 8               # max dilation
W = T + 2 * PAD       # padded activation width
NTW = 512             # matmul free-dim tile (one PSUM bank of f32)
STW = 2048            # psum super-tile (4 banks)
EPS = 1e-5

# 8000 = 3*2048 + 1856: super-tiles of unequal width; use explicit col ranges.
ST_COLS = [(0, 2048), (2048, 4096), (4096, 6144), (6144, 8000)]
NST = len(ST_COLS)    # 4 super-tiles per group
CH = 500              # bn_stats chunk (equal sizes -> exact bn_aggr)
NCH = T // CH         # 16 chunks

VEC_TABLES = ["b1", "g1", "be1", "bd", "swI", "swL", "swR", "g2", "be2"]
VOFF = {t: j * (L * G) for j, t in enumerate(VEC_TABLES)}

LINEARIZE = False   # total-order scheduling (debug)


def _build_program(alphas1, alphas2):
    nc = bacc.Bacc("TRN2", target_bir_lowering=False, debug=False, num_devices=NCORES)

    xbf = nc.dram_tensor("xbf", [128, T], BF16, kind="ExternalInput")
    w1t = nc.dram_tensor("w1t", [128, L * D], BF16, kind="ExternalInput")
    w2t = nc.dram_tensor("w2t", [128, L * D], F32, kind="ExternalInput")
    diag = nc.dram_tensor("diag", [128, L * G * K * 128], BF16, kind="ExternalInput")
    vec = nc.dram_tensor("vec", [128, len(VEC_TABLES) * L * G], F32, kind="ExternalInput")
    b2d = nc.dram_tensor("b2d", [128, L], F32, kind="ExternalInput")
    eye = nc.dram_tensor("eye", [128, 128], BF16, kind="ExternalInput")
    yout = nc.dram_tensor("yout", [128, T], F32, kind="ExternalOutput")

    # collective bounce buffers: warmup pair + one pair per BN.
    # AllReduce(add) of the per-core (mean, q) pairs: the cc cores do the
    # 8-way reduction, so no gather DMA / on-device reduce is needed.
    cins, couts = [], []
    for nm in ["warm"] + [f"{i}_{j}" for i in range(L) for j in range(2)]:
        cins.append(nc.dram_tensor(f"cin_{nm}", [128, 2 * G], F32))
        couts.append(nc.dram_tensor(f"cout_{nm}", [128, 2 * G], F32))

    rgroups = [list(range(NCORES))]

    with tile.TileContext(nc, linearize=LINEARIZE) as tc, ExitStack() as ctx:
        # ---- persistent SBUF ----
        act = [
            nc.alloc_sbuf_tensor(f"act{j}", [128, W], BF16) for j in range(5)
        ]
        w1s = nc.alloc_sbuf_tensor("w1s", [128, L * D], BF16)
        w2s_raw = nc.alloc_sbuf_tensor("w2sraw", [128, L * D], F32)
        vec_s = nc.alloc_sbuf_tensor("vecs", [128, len(VEC_TABLES) * L * G], F32)
        b2_s = nc.alloc_sbuf_tensor("b2s", [128, L], F32)
        eye_s = nc.alloc_sbuf_tensor("eyes", [128, 128], BF16)

        psum = ctx.enter_context(tc.tile_pool(name="psum", bufs=2, space="PSUM"))
        small = ctx.enter_context(tc.tile_pool(name="small", bufs=3))
        diagp = ctx.enter_context(tc.tile_pool(name="diagp", bufs=2))
        stage = ctx.enter_context(tc.tile_pool(name="stage", bufs=3))

        # ---- warmup collective: absorbs the bootstrap barrier + cold start
        wt = small.tile([128, 2 * G], F32, tag="warm")
        nc.vector.memset(wt[:], 0.0)
        nc.sync.dma_start(out=cins[0][:], in_=wt[:])
        nc.gpsimd.collective_compute(
            "AllReduce", ALU.add, replica_groups=rgroups,
            ins=[cins[0][:]], outs=[couts[0][:]],
        )

        # ---- initial loads ----
        nc.sync.dma_start(out=w1s[:], in_=w1t[:])
        nc.sync.dma_start(out=w2s_raw[:], in_=w2t[:])
        nc.sync.dma_start(out=vec_s[:], in_=vec[:])
        nc.sync.dma_start(out=b2_s[:], in_=b2d[:])
        nc.sync.dma_start(out=eye_s[:], in_=eye[:])
        # zero the halo pads of every activation slot
        for a in act:
            nc.vector.memset(a[:, 0:PAD], 0.0)
            nc.vector.memset(a[:, PAD + T : W], 0.0)
        # input x -> act[0] interior (chunked so conv1 can start early)
        for c0 in range(0, T, 2000):
            nc.sync.dma_start(
                out=act[0][:, PAD + c0 : PAD + c0 + 2000], in_=xbf[:, c0 : c0 + 2000]
            )

        def vcol(tbl, i, g=None, n=1):
            off = VOFF[tbl] + i * G + (0 if g is None else g)
            return vec_s[:, off : off + (G if g is None else n)]

        def emit_tile_stats(bnst, src, st, s0, s1c):
            """bn_stats chunks for one supertile of a group, right behind the
            activation that produced it (bn_aggr merges by count, so the
            chunk sizes need not be equal)."""
            wdt = s1c - s0
            chw = (wdt + 3) // 4
            for j in range(4):
                c0 = s0 + j * chw
                c1 = min(c0 + chw, s1c)
                nc.vector.bn_stats(
                    out=bnst[:, 4 * st + j, :], in_=src[:, PAD + c0 : PAD + c1]
                )

        h_idx = 0
        for i in range(L):
            delta = 2 ** i
            a1v = float(alphas1[i])
            a2v = float(alphas2[i])
            h = act[h_idx]
            others = [s for s in range(5) if s != h_idx]
            p1 = [act[s] for s in others]
            p2_idx = [h_idx, others[0], others[1], others[2]]
            p2 = [act[s] for s in p2_idx]
            hn = act[others[3]]

            # layer's diagonal depthwise weights
            dg = diagp.tile([128, G * K * 128], BF16, tag="diag")
            nc.sync.dma_start(
                out=dg[:], in_=diag[:, i * G * K * 128 : (i + 1) * G * K * 128]
            )

            # ---- conv1 (C->D) + PReLU1 + local BN1 stats ----
            pk1 = small.tile([128, 2 * G], F32, tag="pk")
            for g in range(G):
                lw = w1s[:, (i * G + g) * 128 : (i * G + g + 1) * 128]
                bnst = small.tile([128, 4 * NST, 6], F32, tag="bnst")
                for st, (s0, s1c) in enumerate(ST_COLS):
                    ps = psum.tile([128, STW], F32, tag="big")
                    for n0 in range(s0, s1c, NTW):
                        n1 = min(n0 + NTW, s1c)
                        nc.tensor.matmul(
                            ps[:, n0 - s0 : n1 - s0],
                            lw,
                            h[:, PAD + n0 : PAD + n1],
                            start=True,
                            stop=True,
                        )
                    nc.scalar.activation(
                        out=p1[g][:, PAD + s0 : PAD + s1c],
                        in_=ps[:, 0 : s1c - s0],
                        func=AF.Prelu,
                        bias=vcol("b1", i, g),
                        scale=1.0,
                        alpha=a1v,
                    )
                    emit_tile_stats(bnst, p1[g], st, s0, s1c)
                nc.vector.bn_aggr(out=pk1[:, 2 * g : 2 * g + 2], in_=bnst[:])

            # ---- BN1 global stats via AllGather ----
            s1t, t1t = _emit_cross_stats(
                nc, small, pk1, cins[1 + 2 * i], couts[1 + 2 * i],
                rgroups, vcol("g1", i), vcol("be1", i),
            )
            biasI = small.tile([128, G], F32, tag="biasI")
            biasL = small.tile([128, G], F32, tag="biasL")
            biasR = small.tile([128, G], F32, tag="biasR")
            for bt, tbl in ((biasI, "swI"), (biasL, "swL"), (biasR, "swR")):
                nc.vector.tensor_mul(bt[:], t1t[:], vcol(tbl, i))
                nc.vector.tensor_add(bt[:], bt[:], vcol("bd", i))

            # ---- depthwise dilated conv (PE diag matmuls) + PReLU2 + stats ----
            pk2 = small.tile([128, 2 * G], F32, tag="pk")
            for g in range(G):
                bnst = small.tile([128, 4 * NST, 6], F32, tag="bnst")
                for st, (s0, s1c) in enumerate(ST_COLS):
                    ps = psum.tile([128, STW], F32, tag="big")
                    for k in range(K):
                        off = (k - 1) * delta
                        dw = dg[:, (g * K + k) * 128 : (g * K + k + 1) * 128]
                        for n0 in range(s0, s1c, NTW):
                            n1 = min(n0 + NTW, s1c)
                            nc.tensor.matmul(
                                ps[:, n0 - s0 : n1 - s0],
                                dw,
                                p1[g][:, PAD + n0 + off : PAD + n1 + off],
                                start=(k == 0),
                                stop=(k == K - 1),
                            )
                    # PReLU2 with folded BN1 affine; edge columns use
                    # adjusted biases (zero-padding of the BN output).
                    segs = []
                    if st == 0:
                        segs.append((0, delta, biasL))
                        segs.append((delta, s1c - s0, biasI))
                    elif st == NST - 1:
                        segs.append((0, s1c - s0 - delta, biasI))
                        segs.append((s1c - s0 - delta, s1c - s0, biasR))
                    else:
                        segs.append((0, s1c - s0, biasI))
                    for e0, e1, bt in segs:
                        nc.scalar.activation(
                            out=p2[g][:, PAD + s0 + e0 : PAD + s0 + e1],
                            in_=ps[:, e0:e1],
                            func=AF.Prelu,
                            bias=bt[:, g : g + 1],
                            scale=s1t[:, g : g + 1],
                            alpha=a2v,
                        )
                    emit_tile_stats(bnst, p2[g], st, s0, s1c)
                nc.vector.bn_aggr(out=pk2[:, 2 * g : 2 * g + 2], in_=bnst[:])

            # ---- BN2 global stats ----
            s2t, t2t = _emit_cross_stats(
                nc, small, pk2, cins[2 + 2 * i], couts[2 + 2 * i],
                rgroups, vcol("g2", i), vcol("be2", i),
            )

            # ---- fold BN2 into conv2: scale weights to bf16, matvec bias ----
            w2sc = small.tile([128, D], BF16, tag="w2sc")
            for g in range(G):
                nc.vector.tensor_scalar(
                    w2sc[:, g * 128 : (g + 1) * 128],
                    w2s_raw[:, (i * G + g) * 128 : (i * G + g + 1) * 128],
                    s2t[:, g : g + 1],
                    None,
                    ALU.mult,
                )
            mvp = psum.tile([128, STW], F32, tag="big")
            for g in range(G):
                nc.tensor.matmul(
                    mvp[:, 0:1],
                    w2s_raw[:, (i * G + g) * 128 : (i * G + g + 1) * 128],
                    t2t[:, g : g + 1],
                    start=(g == 0),
                    stop=(g == G - 1),
                )
            b2p = small.tile([128, 1], F32, tag="b2p")
            nc.vector.tensor_scalar(
                b2p[:], mvp[:, 0:1], b2_s[:, i : i + 1], None, ALU.add
            )

            # ---- conv2 (D->C) [+ residual x via identity matmul on last layer] ----
            last = i == L - 1
            for st, (s0, s1c) in enumerate(ST_COLS):
                ps = psum.tile([128, STW], F32, tag="big")
                for g in range(G):
                    for n0 in range(s0, s1c, NTW):
                        n1 = min(n0 + NTW, s1c)
                        nc.tensor.matmul(
                            ps[:, n0 - s0 : n1 - s0],
                            w2sc[:, g * 128 : (g + 1) * 128],
                            p2[g][:, PAD + n0 : PAD + n1],
                            start=(g == 0),
                            stop=(g == G - 1 and not last),
                        )
                if last:
                    for n0 in range(s0, s1c, NTW):
                        n1 = min(n0 + NTW, s1c)
                        xs = stage.tile([128, NTW], BF16, tag="xs")
                        nc.sync.dma_start(out=xs[:, 0 : n1 - n0], in_=xbf[:, n0:n1])
                        nc.tensor.matmul(
                            ps[:, n0 - s0 : n1 - s0],
                            eye_s[:],
                            xs[:, 0 : n1 - n0],
                            start=False,
                            stop=True,
                        )
                    yst = stage.tile([128, STW], F32, tag="yst")
                    nc.scalar.activation(
                        out=yst[:, 0 : s1c - s0], in_=ps[:, 0 : s1c - s0],
                        func=AF.Identity, bias=b2p[:], scale=1.0,
                    )
                    nc.sync.dma_start(
                        out=yout[:, s0:s1c], in_=yst[:, 0 : s1c - s0]
                    )
                else:
                    nc.scalar.activation(
                        out=hn[:, PAD + s0 : PAD + s1c], in_=ps[:, 0 : s1c - s0],
                        func=AF.Identity, bias=b2p[:], scale=1.0,
                    )

            h_idx = others[3]

    nc.finalize()
    return nc


def _emit_cross_stats(nc, small, pk, cin, cout, rgroups, gamma, beta):
    """Exchange per-core (mean, mean^2+var) and produce global BN affine.

    pk: [128, 2G] tile with (mean, var) pairs per group from bn_aggr.
    Returns (s, t) tiles [128, G]: s = gamma*rsqrt(var_g+eps),
    t = beta - mean_g*s.
    """
    ev = pk[:, 0 : 2 * G : 2]
    od = pk[:, 1 : 2 * G : 2]
    msq = small.tile([128, G], F32, tag="msq")
    nc.vector.tensor_mul(msq[:], ev, ev)
    nc.vector.tensor_add(od, od, msq[:])  # q = var + mean^2
    nc.sync.dma_start(out=cin[:], in_=pk[:])
    nc.gpsimd.collective_compute(
        "AllReduce", ALU.add, replica_groups=rgroups, ins=[cin[:]], outs=[cout[:]]
    )
    red = small.tile([128, 2 * G], F32, tag="red")
    nc.sync.dma_start(out=red[:], in_=cout[:])
    rev = red[:, 0 : 2 * G : 2]   # sum of means
    rod = red[:, 1 : 2 * G : 2]   # sum of q
    A = small.tile([128, G], F32, tag="A")
    nc.vector.tensor_mul(A[:], rev, rev)  # (sum m)^2
    ve = small.tile([128, G], F32, tag="ve")
    nc.vector.tensor_scalar(ve[:], rod, 1.0 / NCORES, EPS, ALU.mult, ALU.add)
    nc.vector.tensor_scalar(A[:], A[:], 1.0 / (NCORES * NCORES), None, ALU.mult)
    nc.vector.tensor_sub(ve[:], ve[:], A[:])  # var + eps
    sd = small.tile([128, G], F32, tag="sd")
    nc.scalar.activation(out=sd[:], in_=ve[:], func=AF.Sqrt)
    rstd = small.tile([128, G], F32, tag="rstd")
    nc.vector.reciprocal(out=rstd[:], in_=sd[:])
    s = small.tile([128, G], F32, tag="s")
    nc.vector.tensor_mul(s[:], gamma, rstd[:])
    mg = small.tile([128, G], F32, tag="mg")
    nc.vector.tensor_scalar(mg[:], rev, 1.0 / NCORES, None, ALU.mult)
    t = small.tile([128, G], F32, tag="t")
    nc.vector.tensor_mul(t[:], mg[:], s[:])
    nc.vector.tensor_sub(t[:], beta, t[:])
    return s, t


_CACHE = {}


def _get_program(a1, a2):
    key = (tuple(np.asarray(a1, dtype=np.float64)), tuple(np.asarray(a2, dtype=np.float64)))
    if key not in _CACHE:
        _CACHE[key] = _build_program(np.asarray(a1), np.asarray(a2))
    return _CACHE[key]


def _pack_params(w1, b1, g1, be1, wd, bd, g2, be2, w2, b2):
    w1 = np.asarray(w1, np.float32)
    w2 = np.asarray(w2, np.float32)
    wd = np.asarray(wd, np.float32)

    w1t = np.concatenate([w1[i].T for i in range(L)], axis=1)  # [C, L*D]
    # conv2 lhsT block (i,g): [128, 128] with [p, c] = W2[c, g*128+p]
    w2t = np.concatenate(
        [w2[i].T[g * 128 : (g + 1) * 128] for i in range(L) for g in range(G)],
        axis=1,
    )
    assert w2t.shape == (128, L * D)

    dblocks = []
    for i in range(L):
        for g in range(G):
            for k in range(K):
                dblocks.append(np.diag(wd[i, g * 128 : (g + 1) * 128, k]))
    diag = np.concatenate(dblocks, axis=1).astype(np.float32)

    def pack16(tbl):
        # tbl [L, D] -> [128, L*G] with col i*G+g
        out = np.empty((128, L * G), np.float32)
        for i in range(L):
            for g in range(G):
                out[:, i * G + g] = tbl[i, g * 128 : (g + 1) * 128]
        return out

    sw = wd.sum(axis=2)          # [L, D]
    swL = wd[:, :, 1] + wd[:, :, 2]
    swR = wd[:, :, 0] + wd[:, :, 1]
    tables = {
        "b1": pack16(np.asarray(b1, np.float32)),
        "g1": pack16(np.asarray(g1, np.float32)),
        "be1": pack16(np.asarray(be1, np.float32)),
        "bd": pack16(np.asarray(bd, np.float32)),
        "swI": pack16(sw),
        "swL": pack16(swL),
        "swR": pack16(swR),
        "g2": pack16(np.asarray(g2, np.float32)),
        "be2": pack16(np.asarray(be2, np.float32)),
    }
    vec = np.concatenate([tables[t] for t in VEC_TABLES], axis=1)
    b2d = np.asarray(b2, np.float32).T.copy()  # [128, L]
    eye = np.eye(128, dtype=np.float32)
    bf = ml_dtypes.bfloat16
    return {
        "w1t": np.ascontiguousarray(w1t).astype(bf),
        "w2t": np.ascontiguousarray(w2t),
        "diag": np.ascontiguousarray(diag).astype(bf),
        "vec": np.ascontiguousarray(vec),
        "b2d": b2d,
        "eye": eye.astype(bf),
    }


def kernel(x, w1, b1, a1, g1, be1, wd, bd, a2, g2, be2, w2, b2, _trace=False):
    x = np.asarray(x, np.float32)
    nc = _get_program(a1, a2)
    params = _pack_params(w1, b1, g1, be1, wd, bd, g2, be2, w2, b2)
    bf = ml_dtypes.bfloat16
    in_maps = [
        {"xbf": np.ascontiguousarray(x[c]).astype(bf), **params}
        for c in range(NCORES)
    ]
    res = run_bass_kernel_spmd(nc, in_maps, list(range(NCORES)), trace=_trace)
    out = np.stack([res.results[c]["yout"] for c in range(NCORES)], axis=0)
    kernel._last_result = res
    return out.astype(np.float32)


# revision 28
# speedup vs baseline: 1.0674x; 1.0674x over previous
"""Trainium2 Bass kernel for nn_BitwiseTasNetBlock.

Model: 4 layers of [1x1 conv C->D, PReLU, BN, dilated depthwise conv K=3,
PReLU, BN, 1x1 conv D->C] with a residual around the whole stack.
B=8, C=128, D=512, T=8000. Training-mode BatchNorm -> stats over (batch, time).

Sharding: data-parallel over batch, one batch element per NeuronCore (8 cores).

v4 design (bf16 compute):
  - All activations and conv weights are bf16; matmuls run at 1 cycle/row on
    the PE (4x the fp32 rate). PSUM accumulation stays f32.
  - BN stats are exchanged as raw (sum, sumsq) per channel via a small
    AllReduce. The per-channel sum comes free from the PReLU activations'
    accum_out; sumsq comes from affine_mul_reduce ((p*1+0)*p with reduce) on
    DVE - one op per 2048-col supertile, cheaper than 4 bn_stats chunks.
  - A dummy AllReduce at kernel start absorbs the collectives bootstrap
    barrier (~45us) and the cold-start cost of the first real exchange.
  - BN1 affine folds into the PReLU2 activation (scale/bias APs); depthwise
    edge columns use per-edge bias variants. BN2 folds into conv2 weights
    (scaled to bf16 on device) + bias (W2 @ t2 matvec).
  - The depthwise outputs of groups 0-1 are drained PSUM->SBUF (scalar copy,
    during the scalar lull while the BN1 exchange is in flight) so the PE
    can keep running depthwise matmuls instead of stalling on PSUM back-
    pressure; their PReLU2 then reads from SBUF.
  - The residual is an identity matmul emitted before the conv2 accumulation
    on the last layer, so it runs during the BN2 exchange.
"""

import numpy as np
from contextlib import ExitStack

import ml_dtypes

import concourse.bass as bass
import concourse.bacc as bacc
import concourse.mybir as mybir
import concourse.tile as tile
from concourse.bass_utils import run_bass_kernel_spmd

F32 = mybir.dt.float32
BF16 = mybir.dt.bfloat16
AF = mybir.ActivationFunctionType
ALU = mybir.AluOpType

NCORES = 8
B, C, D, T, L, K = 8, 128, 512, 8000, 4, 3
G = D // 128          # 4 channel groups of 128 partitions
PAD = 8               # max dilation
W = T + 2 * PAD       # padded activation width
NTW = 512             # matmul free-dim tile (one PSUM bank of f32)
STW = 2048            # psum super-tile (4 banks)
EPS = 1e-5
NT_TOTAL = float(NCORES * T)   # BN sample count

# 8000 = 3*2048 + 1856: super-tiles of unequal width; use explicit col ranges.
ST_COLS = [(0, 2048), (2048, 4096), (4096, 6144), (6144, 8000)]
NST = len(ST_COLS)    # 4 super-tiles per group
NST2 = NST + 2        # PReLU2 sum slots (st0 and st3 have an edge segment)

NDRAIN = 2            # depthwise groups drained PSUM->SBUF (0..NDRAIN-1)

VEC_TABLES = ["b1", "g1", "be1", "bd", "swI", "swL", "swR", "g2", "be2"]
VOFF = {t: j * (L * G) for j, t in enumerate(VEC_TABLES)}

LINEARIZE = False   # total-order scheduling (debug)


def _build_program(alphas1, alphas2):
    nc = bacc.Bacc("TRN2", target_bir_lowering=False, debug=False, num_devices=NCORES)

    xbf = nc.dram_tensor("xbf", [128, T], BF16, kind="ExternalInput")
    w1t = nc.dram_tensor("w1t", [128, L * D], BF16, kind="ExternalInput")
    w2t = nc.dram_tensor("w2t", [128, L * D], F32, kind="ExternalInput")
    diag = nc.dram_tensor("diag", [128, L * G * K * 128], BF16, kind="ExternalInput")
    vec = nc.dram_tensor("vec", [128, len(VEC_TABLES) * L * G], F32, kind="ExternalInput")
    b2d = nc.dram_tensor("b2d", [128, L], F32, kind="ExternalInput")
    eye = nc.dram_tensor("eye", [128, 128], BF16, kind="ExternalInput")
    yout = nc.dram_tensor("yout", [128, T], F32, kind="ExternalOutput")

    # collective bounce buffers: warmup pair + one pair per BN.
    # AllReduce(add) of per-core (sum, sumsq): the cc cores do the 8-way
    # reduction, so no gather DMA / on-device reduce is needed.
    cins, couts = [], []
    for nm in ["warm"] + [f"{i}_{j}" for i in range(L) for j in range(2)]:
        cins.append(nc.dram_tensor(f"cin_{nm}", [128, 2 * G], F32))
        couts.append(nc.dram_tensor(f"cout_{nm}", [128, 2 * G], F32))

    rgroups = [list(range(NCORES))]

    with tile.TileContext(nc, linearize=LINEARIZE) as tc, ExitStack() as ctx:
        # ---- persistent SBUF ----
        act = [
            nc.alloc_sbuf_tensor(f"act{j}", [128, W], BF16) for j in range(5)
        ]
        qbuf = [
            nc.alloc_sbuf_tensor(f"qbuf{j}", [128, T], BF16) for j in range(NDRAIN)
        ]
        w1s = nc.alloc_sbuf_tensor("w1s", [128, L * D], BF16)
        w2s_raw = nc.alloc_sbuf_tensor("w2sraw", [128, L * D], F32)
        vec_s = nc.alloc_sbuf_tensor("vecs", [128, len(VEC_TABLES) * L * G], F32)
        b2_s = nc.alloc_sbuf_tensor("b2s", [128, L], F32)
        eye_s = nc.alloc_sbuf_tensor("eyes", [128, 128], BF16)

        psum = ctx.enter_context(tc.tile_pool(name="psum", bufs=2, space="PSUM"))
        small = ctx.enter_context(tc.tile_pool(name="small", bufs=3))
        diagp = ctx.enter_context(tc.tile_pool(name="diagp", bufs=2))
        stage = ctx.enter_context(tc.tile_pool(name="stage", bufs=3))
        sqp = ctx.enter_context(tc.tile_pool(name="sqp", bufs=2))

        # ---- warmup collective: absorbs the bootstrap barrier + cold start
        wt = small.tile([128, 2 * G], F32, tag="warm")
        nc.vector.memset(wt[:], 0.0)
        nc.sync.dma_start(out=cins[0][:], in_=wt[:])
        nc.gpsimd.collective_compute(
            "AllReduce", ALU.add, replica_groups=rgroups,
            ins=[cins[0][:]], outs=[couts[0][:]],
        )

        # ---- initial loads ----
        nc.sync.dma_start(out=w1s[:], in_=w1t[:])
        nc.sync.dma_start(out=w2s_raw[:], in_=w2t[:])
        nc.sync.dma_start(out=vec_s[:], in_=vec[:])
        nc.sync.dma_start(out=b2_s[:], in_=b2d[:])
        nc.sync.dma_start(out=eye_s[:], in_=eye[:])
        # zero the halo pads of every activation slot
        for a in act:
            nc.vector.memset(a[:, 0:PAD], 0.0)
            nc.vector.memset(a[:, PAD + T : W], 0.0)
        # input x -> act[0] interior (chunked so conv1 can start early)
        for c0 in range(0, T, 2000):
            nc.sync.dma_start(
                out=act[0][:, PAD + c0 : PAD + c0 + 2000], in_=xbf[:, c0 : c0 + 2000]
            )

        def vcol(tbl, i, g=None, n=1):
            off = VOFF[tbl] + i * G + (0 if g is None else g)
            return vec_s[:, off : off + (G if g is None else n)]

        def emit_sumsq(src_ap, qacc_col):
            """sumsq of one supertile via affine_mul_reduce on DVE."""
            sq = sqp.tile([128, STW], F32, tag="sq")
            w_ = src_ap.shape[-1]
            nc.vector.affine_mul_reduce(
                out=sq[:, 0:w_], accum_out=qacc_col,
                in0=src_ap, in1=src_ap, scale=1.0, bias=0.0,
            )

        h_idx = 0
        for i in range(L):
            delta = 2 ** i
            a1v = float(alphas1[i])
            a2v = float(alphas2[i])
            h = act[h_idx]
            others = [s for s in range(5) if s != h_idx]
            p1 = [act[s] for s in others]
            p2_idx = [h_idx, others[0], others[1], others[2]]
            p2 = [act[s] for s in p2_idx]
            hn = act[others[3]]

            # layer's diagonal depthwise weights
            dg = diagp.tile([128, G * K * 128], BF16, tag="diag")
            nc.sync.dma_start(
                out=dg[:], in_=diag[:, i * G * K * 128 : (i + 1) * G * K * 128]
            )

            # ---- conv1 (C->D) + PReLU1 (accum -> sum) + sumsq ----
            acc1 = small.tile([128, G, NST], F32, tag="acc1")
            qacc1 = small.tile([128, G, NST], F32, tag="qacc1")
            for g in range(G):
                lw = w1s[:, (i * G + g) * 128 : (i * G + g + 1) * 128]
                for st, (s0, s1c) in enumerate(ST_COLS):
                    ps = psum.tile([128, STW], F32, tag="big")
                    for n0 in range(s0, s1c, NTW):
                        n1 = min(n0 + NTW, s1c)
                        nc.tensor.matmul(
                            ps[:, n0 - s0 : n1 - s0],
                            lw,
                            h[:, PAD + n0 : PAD + n1],
                            start=True,
                            stop=True,
                        )
                    nc.scalar.activation(
                        out=p1[g][:, PAD + s0 : PAD + s1c],
                        in_=ps[:, 0 : s1c - s0],
                        func=AF.Prelu,
                        bias=vcol("b1", i, g),
                        scale=1.0,
                        alpha=a1v,
                        accum_out=acc1[:, g, st : st + 1],
                    )
                    emit_sumsq(
                        p1[g][:, PAD + s0 : PAD + s1c], qacc1[:, g, st : st + 1]
                    )

            # ---- BN1 global stats via AllReduce of (sum, sumsq) ----
            s1t, t1t = _emit_stats_exchange(
                nc, small, acc1, qacc1, cins[1 + 2 * i], couts[1 + 2 * i],
                rgroups, vcol("g1", i), vcol("be1", i),
            )
            biasI = small.tile([128, G], F32, tag="biasI")
            biasL = small.tile([128, G], F32, tag="biasL")
            biasR = small.tile([128, G], F32, tag="biasR")
            for bt, tbl in ((biasI, "swI"), (biasL, "swL"), (biasR, "swR")):
                nc.vector.tensor_mul(bt[:], t1t[:], vcol(tbl, i))
                nc.vector.tensor_add(bt[:], bt[:], vcol("bd", i))

            # ---- depthwise dilated conv (PE diag matmuls) ----
            # Groups < NDRAIN: PSUM is drained to SBUF immediately (no s1t
            # dependency) so the PE keeps streaming during the exchange.
            acc2 = small.tile([128, G, NST2], F32, tag="acc2")
            qacc2 = small.tile([128, G, NST], F32, tag="qacc2")

            def p2_segs(st, s0, s1c):
                # Edge columns use adjusted biases (zero-padding of the BN
                # output feeding the depthwise conv).
                if st == 0:
                    return [(0, delta, biasL, NST), (delta, s1c - s0, biasI, st)]
                if st == NST - 1:
                    return [(0, s1c - s0 - delta, biasI, st),
                            (s1c - s0 - delta, s1c - s0, biasR, NST + 1)]
                return [(0, s1c - s0, biasI, st)]

            def emit_prelu2(g, st, s0, s1c, src_tile, src_off):
                for e0, e1, bt, slot in p2_segs(st, s0, s1c):
                    nc.scalar.activation(
                        out=p2[g][:, PAD + s0 + e0 : PAD + s0 + e1],
                        in_=src_tile[:, src_off + e0 : src_off + e1],
                        func=AF.Prelu,
                        bias=bt[:, g : g + 1],
                        scale=s1t[:, g : g + 1],
                        alpha=a2v,
                        accum_out=acc2[:, g, slot : slot + 1],
                    )
                emit_sumsq(
                    p2[g][:, PAD + s0 : PAD + s1c], qacc2[:, g, st : st + 1]
                )

            for g in range(G):
                for st, (s0, s1c) in enumerate(ST_COLS):
                    ps = psum.tile([128, STW], F32, tag="big")
                    for k in range(K):
                        off = (k - 1) * delta
                        dwt = dg[:, (g * K + k) * 128 : (g * K + k + 1) * 128]
                        for n0 in range(s0, s1c, NTW):
                            n1 = min(n0 + NTW, s1c)
                            nc.tensor.matmul(
                                ps[:, n0 - s0 : n1 - s0],
                                dwt,
                                p1[g][:, PAD + n0 + off : PAD + n1 + off],
                                start=(k == 0),
                                stop=(k == K - 1),
                            )
                    if g < NDRAIN:
                        # drain to SBUF now (no s1t dependency); PReLU2 for
                        # this group reads qbuf later
                        nc.scalar.copy(
                            out=qbuf[g][:, s0:s1c], in_=ps[:, 0 : s1c - s0]
                        )
                    else:
                        emit_prelu2(g, st, s0, s1c, ps, 0)
            for g in range(NDRAIN):
                for st, (s0, s1c) in enumerate(ST_COLS):
                    emit_prelu2(g, st, s0, s1c, qbuf[g], s0)

            # ---- BN2 global stats ----
            s2t, t2t = _emit_stats_exchange(
                nc, small, acc2, qacc2, cins[2 + 2 * i], couts[2 + 2 * i],
                rgroups, vcol("g2", i), vcol("be2", i),
            )

            # ---- fold BN2 into conv2: scale weights to bf16, matvec bias ----
            w2sc = small.tile([128, D], BF16, tag="w2sc")
            for g in range(G):
                nc.vector.tensor_scalar(
                    w2sc[:, g * 128 : (g + 1) * 128],
                    w2s_raw[:, (i * G + g) * 128 : (i * G + g + 1) * 128],
                    s2t[:, g : g + 1],
                    None,
                    ALU.mult,
                )
            mvp = psum.tile([128, STW], F32, tag="big")
            for g in range(G):
                nc.tensor.matmul(
                    mvp[:, 0:1],
                    w2s_raw[:, (i * G + g) * 128 : (i * G + g + 1) * 128],
                    t2t[:, g : g + 1],
                    start=(g == 0),
                    stop=(g == G - 1),
                )
            b2p = small.tile([128, 1], F32, tag="b2p")
            nc.vector.tensor_scalar(
                b2p[:], mvp[:, 0:1], b2_s[:, i : i + 1], None, ALU.add
            )

            # ---- conv2 (D->C) [+ residual x via identity matmul on last layer] ----
            last = i == L - 1
            for st, (s0, s1c) in enumerate(ST_COLS):
                ps = psum.tile([128, STW], F32, tag="big")
                if last:
                    # residual first: no s2 dependency, runs during the
                    # BN2 exchange
                    for n0 in range(s0, s1c, NTW):
                        n1 = min(n0 + NTW, s1c)
                        xs = stage.tile([128, NTW], BF16, tag="xs")
                        nc.sync.dma_start(out=xs[:, 0 : n1 - n0], in_=xbf[:, n0:n1])
                        nc.tensor.matmul(
                            ps[:, n0 - s0 : n1 - s0],
                            eye_s[:],
                            xs[:, 0 : n1 - n0],
                            start=True,
                            stop=False,
                        )
                for g in range(G):
                    for n0 in range(s0, s1c, NTW):
                        n1 = min(n0 + NTW, s1c)
                        nc.tensor.matmul(
                            ps[:, n0 - s0 : n1 - s0],
                            w2sc[:, g * 128 : (g + 1) * 128],
                            p2[g][:, PAD + n0 : PAD + n1],
                            start=(g == 0 and not last),
                            stop=(g == G - 1),
                        )
                if last:
                    yst = stage.tile([128, STW], F32, tag="yst")
                    nc.scalar.activation(
                        out=yst[:, 0 : s1c - s0], in_=ps[:, 0 : s1c - s0],
                        func=AF.Identity, bias=b2p[:], scale=1.0,
                    )
                    nc.sync.dma_start(
                        out=yout[:, s0:s1c], in_=yst[:, 0 : s1c - s0]
                    )
                else:
                    nc.scalar.activation(
                        out=hn[:, PAD + s0 : PAD + s1c], in_=ps[:, 0 : s1c - s0],
                        func=AF.Identity, bias=b2p[:], scale=1.0,
                    )

            h_idx = others[3]

    nc.finalize()
    return nc


def _emit_stats_exchange(nc, small, acc, qacc, cin, cout, rgroups, gamma, beta):
    """AllReduce per-core (sum, sumsq) and produce the global BN affine.

    acc: [128, G, nslots] partial sums (activation accum_out per tile).
    qacc: [128, G, NST] partial sumsqs (affine_mul_reduce per tile).
    Returns (s, t) tiles [128, G]: s = gamma*rsqrt(var_g+eps),
    t = beta - mean_g*s.
    """
    pk = small.tile([128, 2 * G], F32, tag="pk")
    for g in range(G):
        nc.vector.tensor_reduce(
            out=pk[:, 2 * g : 2 * g + 1], in_=acc[:, g, :],
            axis=mybir.AxisListType.X, op=ALU.add,
        )
        nc.vector.tensor_reduce(
            out=pk[:, 2 * g + 1 : 2 * g + 2], in_=qacc[:, g, :],
            axis=mybir.AxisListType.X, op=ALU.add,
        )
    nc.sync.dma_start(out=cin[:], in_=pk[:])
    nc.gpsimd.collective_compute(
        "AllReduce", ALU.add, replica_groups=rgroups, ins=[cin[:]], outs=[cout[:]]
    )
    red = small.tile([128, 2 * G], F32, tag="red")
    nc.sync.dma_start(out=red[:], in_=cout[:])
    rev = red[:, 0 : 2 * G : 2]   # global sum
    rod = red[:, 1 : 2 * G : 2]   # global sumsq
    # s first: conv2's weight scaling only needs s, so it can start sooner
    mg = small.tile([128, G], F32, tag="mg")
    nc.vector.tensor_scalar(mg[:], rev, 1.0 / NT_TOTAL, None, ALU.mult)
    ve = small.tile([128, G], F32, tag="ve")
    nc.vector.tensor_scalar(ve[:], rod, 1.0 / NT_TOTAL, EPS, ALU.mult, ALU.add)
    A = small.tile([128, G], F32, tag="A")
    nc.vector.tensor_mul(A[:], mg[:], mg[:])
    nc.vector.tensor_sub(ve[:], ve[:], A[:])  # var + eps
    sd = small.tile([128, G], F32, tag="sd")
    nc.scalar.activation(out=sd[:], in_=ve[:], func=AF.Sqrt)
    rstd = small.tile([128, G], F32, tag="rstd")
    nc.vector.reciprocal(out=rstd[:], in_=sd[:])
    s = small.tile([128, G], F32, tag="s")
    nc.vector.tensor_mul(s[:], gamma, rstd[:])
    t = small.tile([128, G], F32, tag="t")
    nc.vector.tensor_mul(t[:], mg[:], s[:])
    nc.vector.tensor_sub(t[:], beta, t[:])
    return s, t


_CACHE = {}


def _get_program(a1, a2):
    key = (tuple(np.asarray(a1, dtype=np.float64)), tuple(np.asarray(a2, dtype=np.float64)))
    if key not in _CACHE:
        _CACHE[key] = _build_program(np.asarray(a1), np.asarray(a2))
    return _CACHE[key]


def _pack_params(w1, b1, g1, be1, wd, bd, g2, be2, w2, b2):
    w1 = np.asarray(w1, np.float32)
    w2 = np.asarray(w2, np.float32)
    wd = np.asarray(wd, np.float32)

    w1t = np.concatenate([w1[i].T for i in range(L)], axis=1)  # [C, L*D]
    # conv2 lhsT block (i,g): [128, 128] with [p, c] = W2[c, g*128+p]
    w2t = np.concatenate(
        [w2[i].T[g * 128 : (g + 1) * 128] for i in range(L) for g in range(G)],
        axis=1,
    )
    assert w2t.shape == (128, L * D)

    dblocks = []
    for i in range(L):
        for g in range(G):
            for k in range(K):
                dblocks.append(np.diag(wd[i, g * 128 : (g + 1) * 128, k]))
    diag = np.concatenate(dblocks, axis=1).astype(np.float32)

    def pack16(tbl):
        # tbl [L, D] -> [128, L*G] with col i*G+g
        out = np.empty((128, L * G), np.float32)
        for i in range(L):
            for g in range(G):
                out[:, i * G + g] = tbl[i, g * 128 : (g + 1) * 128]
        return out

    sw = wd.sum(axis=2)          # [L, D]
    swL = wd[:, :, 1] + wd[:, :, 2]
    swR = wd[:, :, 0] + wd[:, :, 1]
    tables = {
        "b1": pack16(np.asarray(b1, np.float32)),
        "g1": pack16(np.asarray(g1, np.float32)),
        "be1": pack16(np.asarray(be1, np.float32)),
        "bd": pack16(np.asarray(bd, np.float32)),
        "swI": pack16(sw),
        "swL": pack16(swL),
        "swR": pack16(swR),
        "g2": pack16(np.asarray(g2, np.float32)),
        "be2": pack16(np.asarray(be2, np.float32)),
    }
    vec = np.concatenate([tables[t] for t in VEC_TABLES], axis=1)
    b2d = np.asarray(b2, np.float32).T.copy()  # [128, L]
    eye = np.eye(128, dtype=np.float32)
    bf = ml_dtypes.bfloat16
    return {
        "w1t": np.ascontiguousarray(w1t).astype(bf),
        "w2t": np.ascontiguousarray(w2t),
        "diag": np.ascontiguousarray(diag).astype(bf),
        "vec": np.ascontiguousarray(vec),
        "b2d": b2d,
        "eye": eye.astype(bf),
    }


def kernel(x, w1, b1, a1, g1, be1, wd, bd, a2, g2, be2, w2, b2, _trace=False):
    x = np.asarray(x, np.float32)
    nc = _get_program(a1, a2)
    params = _pack_params(w1, b1, g1, be1, wd, bd, g2, be2, w2, b2)
    bf = ml_dtypes.bfloat16
    in_maps = [
        {"xbf": np.ascontiguousarray(x[c]).astype(bf), **params}
        for c in range(NCORES)
    ]
    res = run_bass_kernel_spmd(nc, in_maps, list(range(NCORES)), trace=_trace)
    out = np.stack([res.results[c]["yout"] for c in range(NCORES)], axis=0)
    kernel._last_result = res
    return out.astype(np.float32)
